# revision 20
# baseline (speedup 1.0000x reference)
"""MAGNO encoder kernel for 8 Trainium2 NeuronCores — v2.

Strategy (dst-sharded, slot-aligned edge grid, fused activations, no gather):
  - Latents are assigned to (core, bucket, slot) on the host, balanced by
    edge count (count-sorted slices of 128, snake-assigned over cores).
    Core c owns NBKT buckets of 128 latent slots; output rows are
    inverse-permuted on the host afterwards.
  - Edges are laid out on the host in a [128 slots x ncols] grid per bucket:
    grid column j is one 128-edge chunk whose slot-p edge has dst == slot p.
    Scatter therefore needs NO per-edge one-hot: chunk tiles are accumulated
    into G[slot, hid] with plain vector adds. Padding slots stream all-zero
    features and contribute exactly 0 (b1 = b2 = 0 per the model spec).
  - Per-edge 10-dim fp16 inputs [f_src, p_src, latent_pos[dst], 0] are packed
    on the host (pure index gather) into a [10, ne] stream: 20 B/edge of HBM
    traffic, no dma_gather, no GpSimd work at all.
  - Algebra: edge_in @ W1 = [f, p, latpos_dst] @ [W1_f; W1_p - W1_rel;
    W1_rel]; W3 is applied after aggregation (it is linear).
  - gelu1(h1 of superchunk k) and gelu2(a2 of superchunk k-2) are fused into
    ONE scalar-engine ACTIVATE over a contiguous [128, 2048] PSUM region,
    amortizing the ~352-cycle ACT fixed overhead. The k-2 pairing keeps the
    scalar engine (the critical resource: 512 gelu evals/edge) bubble-free:
    PSUM region r = k%2 holds [h1(k) | a2(k-2)], and L2(k) overwrites the a2
    section right after ACT(k) reads it.
  - PSUM: exactly two [128, 8, 256] f32 regions = 8 banks.
  - PE program order is software-pipelined (L1(k+2) emitted before L2(k)) so
    the tensor engine never blocks the activation chain.
  - Epilogue per bucket: O = (G @ diag(rcnt))^T-via-matmul @ W3 + b3; the
    1/cnt scaling rides along the transpose matmul using a host-built
    diagonal matrix (counts are index-derived host data).
"""

import os
import numpy as np

import concourse.bass as bass
import concourse.mybir as mybir
import concourse.tile as tile
from concourse import bacc
from concourse.bass_utils import run_bass_kernel_spmd

P = 128
N_PHYS = 100000
N_LATENT = 4096
HID = 256
NCORES = 8
NBKT = 4                      # buckets (slices of 128 latent slots) per core
SUP = 512                     # superchunk edge count (4 chunks of 128)
NCH = SUP // P                # chunks per superchunk = 4
DMACH = 16                    # superchunks per input DMA batch

f32 = mybir.dt.float32
f16 = mybir.dt.float16

last_results = None  # set by kernel(); test harness reads exec_time_ns


def _build_program(supb, b3nz):
    """supb[b]: superchunks in bucket b (same for all cores)."""
    nsup = sum(supb)
    ne = nsup * SUP
    # bucket of superchunk k
    kbkt = []
    for b in range(NBKT):
        kbkt += [b] * supb[b]

    nc = bacc.Bacc("TRN2", target_bir_lowering=False)

    xtr_d = nc.dram_tensor("xtr", [10, ne], f16, kind="ExternalInput")
    W1f_d = nc.dram_tensor("W1f", [10, HID], f16, kind="ExternalInput")
    W2v_d = nc.dram_tensor("W2v", [P, 2 * HID], f16, kind="ExternalInput")
    W3p_d = nc.dram_tensor("W3p", [P, 2 * HID], f16, kind="ExternalInput")
    rdiag_d = nc.dram_tensor("rdiag", [P, NBKT * P], f16, kind="ExternalInput")
    b3_d = nc.dram_tensor("b3r", [1, HID], f32, kind="ExternalInput")
    ones1_d = nc.dram_tensor("ones1", [1, P], f32, kind="ExternalInput")
    out_d = nc.dram_tensor("out", [NBKT * P, HID], f32, kind="ExternalOutput")

    GELU = mybir.ActivationFunctionType.Gelu_apprx_tanh

    with tile.TileContext(nc) as tc:
        with tc.tile_pool(name="const", bufs=1) as cp:

            wu_sb = cp.tile([P, 2, HID], f16, tag="wu_sb")
            nc.vector.memset(wu_sb[:], 1.0)

            def load(shape, dt, src_ap, tag, eng=None):
                t = cp.tile(shape, dt, tag=tag)
                (eng or nc.default_dma_engine).dma_start(out=t[:], in_=src_ap)
                return t

            # main-path weights ride the sync queue (with the xtr stream);
            # epilogue-only tensors go on the default queue in parallel.
            W1f_t = load([10, HID], f16, W1f_d[:], "W1f", nc.sync)
            W2v_t = load([P, 2, HID], f16, W2v_d[:], "W2v", nc.sync)
            W3p_t = load([P, 2, HID], f16, W3p_d[:], "W3p")
            rdiag_t = load([P, NBKT, P], f16, rdiag_d[:], "rdiag")
            if b3nz:
                b3_t = load([1, HID], f32, b3_d[:], "b3")
                ones1_t = load([1, P], f32, ones1_d[:], "ones1")

            # persistent SBUF accumulators per bucket, zeroed
            Gsb = [cp.tile([P, HID], f32, tag=f"Gsb{b}", name=f"Gsb{b}")
                   for b in range(NBKT)]
            for b in range(NBKT):
                nc.gpsimd.memset(Gsb[b][:], 0.0)

            with tc.tile_pool(name="psR", bufs=1, space="PSUM") as psR, \
                 tc.tile_pool(name="xin", bufs=3) as xp, \
                 tc.tile_pool(name="act", bufs=3) as ap_, \
                 tc.tile_pool(name="sc", bufs=3) as sp:

                # PSUM ping-pong regions: [h1 (2x512) | a2 (4x256)] = 4 banks
                REG = [psR.tile([P, 8, 256], f32, tag=f"REG{x}",
                                name=f"REG{x}") for x in range(2)]

                # PE warm-up: ~4us of back-to-back matmuls flips the HAM
                # clock gate from 4/8 (1.2 GHz) to 8/8 (2.4 GHz); the loop's
                # filler matmuls then keep it warm. Targets the h1 area that
                # L1(0) overwrites, so no extra PSUM and no pool barrier.
                for _ in range(9):
                    nc.tensor.matmul(out=REG[0][:, 0:2, :],
                                     lhsT=wu_sb[:, 0, 0:P], rhs=wu_sb[:],
                                     start=True, stop=True,
                                     skip_group_check=True)

                xt_tiles = {}

                def ensure_xt(k):
                    """Emit the DMA for the batch containing superchunk k."""
                    nb = k // DMACH
                    if nb in xt_tiles or k >= nsup:
                        return
                    nsw = min(DMACH, nsup - nb * DMACH)
                    t = xp.tile([10, DMACH, SUP], f16, tag="xt")
                    nc.sync.dma_start(
                        out=t[:, :nsw, :],
                        in_=xtr_d[:, nb * DMACH * SUP:
                                  (nb * DMACH + nsw) * SUP])
                    xt_tiles[nb] = t
                    xt_tiles.pop(nb - 2, None)

                def emit_L1(k):
                    if k >= nsup:
                        return
                    ensure_xt(k)
                    r = k % 2
                    xt = xt_tiles[k // DMACH]
                    for m in range(2):
                        nc.tensor.matmul(
                            out=REG[r][:, 2 * m:2 * m + 2, :],
                            lhsT=W1f_t[:, m * P:(m + 1) * P],
                            rhs=xt[:, k % DMACH, :],
                            start=True, stop=True,
                            skip_group_check=True)

                def emit_DVE_scatter(ao, ksrc):
                    """Fold a2h(ksrc) (= ao[:, 4:8, :]) into Gsb[bucket]."""
                    b = kbkt[ksrc]
                    s1 = sp.tile([P, HID], f16, tag="s1")
                    nc.vector.tensor_tensor(out=s1[:], in0=ao[:, 4, :],
                                            in1=ao[:, 5, :],
                                            op=mybir.AluOpType.add)
                    s2 = sp.tile([P, HID], f16, tag="s2")
                    nc.vector.tensor_tensor(out=s2[:], in0=ao[:, 6, :],
                                            in1=ao[:, 7, :],
                                            op=mybir.AluOpType.add)
                    s3 = sp.tile([P, HID], f16, tag="s3")
                    nc.vector.tensor_tensor(out=s3[:], in0=s1[:], in1=s2[:],
                                            op=mybir.AluOpType.add)
                    s4 = sp.tile([P, HID], f32, tag="s4")
                    nc.vector.tensor_copy(out=s4[:], in_=s3[:])
                    nc.vector.tensor_tensor(out=Gsb[b][:], in0=Gsb[b][:],
                                            in1=s4[:],
                                            op=mybir.AluOpType.add)

                ndummy = int(os.environ.get("MAGNO_DUMMY", "2"))
                ndldw = int(os.environ.get("MAGNO_DLDW", "0"))
                emit_L1(0)
                emit_L1(1)
                for k in range(nsup):
                    r = k % 2
                    # fused ACT(k): gelu over [h1(k) | a2(k-2)]
                    ao = ap_.tile([P, 8, 256], f16, tag="ao")
                    if k >= 2:
                        nc.scalar.activation(out=ao[:], in_=REG[r][:],
                                             func=GELU)
                    else:
                        nc.scalar.activation(out=ao[:, 0:4, :],
                                             in_=REG[r][:, 0:4, :], func=GELU)
                    # PE filler: dummy matmuls into the h1 area that L1
                    # overwrites right after; enough PE duty to keep the HAM
                    # clock gate at 8/8 (2.4 GHz).
                    for d in range(ndummy):
                        nc.tensor.matmul(out=REG[r][:, 0:2 - d, :],
                                         lhsT=wu_sb[:, 0, 0:P],
                                         rhs=wu_sb[:] if d == 0
                                         else wu_sb[:, 0, :],
                                         start=True, stop=True,
                                         skip_group_check=True)
                    # PE: L1(k+2) first, then L2(k) (keeps ACT chain fed)
                    emit_L1(k + 2)
                    ensure_xt(k + 2 + DMACH)
                    for j in range(NCH):
                        q = j // 2
                        o = (j % 2) * P
                        for m in range(2):
                            nc.tensor.matmul(
                                out=REG[r][:, 4 + j, :],
                                lhsT=ao[:, 2 * m + q, o:o + P],
                                rhs=W2v_t[:, m, :],
                                start=(m == 0), stop=(m == 1),
                                skip_group_check=True)
                    # PE filler (off-chain): dummy LDWEIGHTS after the real
                    # block keep the HAM clock gate at 8/8 (2.4 GHz) without
                    # touching PSUM or sitting on the ACT dependency chain.
                    for _ in range(ndldw):
                        nc.tensor.ldweights(weights=W2v_t[:, 0, 0:P])
                    # DVE: scatter a2h(k-2)
                    if k >= 2:
                        emit_DVE_scatter(ao, k - 2)

                # tails: a2(nsup-2), a2(nsup-1) still in PSUM
                for ktail in (nsup - 2, nsup - 1):
                    r = ktail % 2
                    ao = ap_.tile([P, 8, 256], f16, tag="ao")
                    nc.scalar.activation(out=ao[:, 4:8, :],
                                         in_=REG[r][:, 4:8, :], func=GELU)
                    emit_DVE_scatter(ao, ktail)

            # ---- epilogue: O = (Gsb @ diag(rcnt))^T @ W3 (+ b3) ----
            with tc.tile_pool(name="ep", bufs=2) as ep, \
                 tc.tile_pool(name="psE", bufs=2, space="PSUM") as psE:
                if b3nz:
                    b3_ps = psE.tile([P, HID], f32, tag="b3bc")
                    nc.tensor.matmul(out=b3_ps[:], lhsT=ones1_t[:], rhs=b3_t[:],
                                     start=True, stop=True)
                    b3bc_t = ep.tile([P, HID], f32, tag="b3bc")
                    nc.vector.tensor_copy(out=b3bc_t[:], in_=b3_ps[:])
                for b in range(NBKT):
                    # gt[h, l] = Gsb[l, h] * rcnt[l] via matmul with diag
                    gsh_t = ep.tile([P, HID], f16, tag="gsh")
                    nc.vector.tensor_copy(out=gsh_t[:], in_=Gsb[b][:])
                    gth_t = ep.tile([P, 2, P], f16, tag="gth")
                    for q in range(2):
                        gt_ps = psE.tile([P, P], f32, tag="gt")
                        nc.tensor.matmul(out=gt_ps[:],
                                         lhsT=gsh_t[:, q * P:(q + 1) * P],
                                         rhs=rdiag_t[:, b, :],
                                         start=True, stop=True)
                        nc.vector.tensor_copy(out=gth_t[:, q, :], in_=gt_ps[:])
                    o_ps = psE.tile([P, HID], f32, tag="o")
                    nc.tensor.matmul(out=o_ps[:], lhsT=gth_t[:, 0, :],
                                     rhs=W3p_t[:, 0, :], start=True, stop=False)
                    nc.tensor.matmul(out=o_ps[:], lhsT=gth_t[:, 1, :],
                                     rhs=W3p_t[:, 1, :], start=False, stop=True)
                    o_t = ep.tile([P, HID], f32, tag="osb")
                    if b3nz:
                        nc.vector.tensor_tensor(out=o_t[:], in0=o_ps[:],
                                                in1=b3bc_t[:],
                                                op=mybir.AluOpType.add)
                    else:
                        nc.vector.tensor_copy(out=o_t[:], in_=o_ps[:])
                    nc.default_dma_engine.dma_start(
                        out=out_d[b * P:(b + 1) * P, :], in_=o_t[:])

    nc.finalize()
    return nc


def kernel(phys_feats, phys_pos, latent_pos, edge_src, edge_dst,
           W1, b1, W2, b2, W3, b3):
    global last_results
    phys_feats = np.asarray(phys_feats, dtype=np.float32)
    phys_pos = np.asarray(phys_pos, dtype=np.float32)
    latent_pos = np.asarray(latent_pos, dtype=np.float32)
    W1 = np.asarray(W1, dtype=np.float32)
    W2 = np.asarray(W2, dtype=np.float32)
    W3 = np.asarray(W3, dtype=np.float32)
    b1 = np.asarray(b1, dtype=np.float32)
    b2 = np.asarray(b2, dtype=np.float32)
    b3 = np.asarray(b3, dtype=np.float32)
    src_all = np.asarray(edge_src).reshape(-1).astype(np.int64)
    dst_all = np.asarray(edge_dst).reshape(-1).astype(np.int64)
    E = src_all.shape[0]
    assert not b1.any() and not b2.any(), "zero b1/b2 assumed (spec fill=zeros)"

    # ---- latent -> (core, bucket, slot) balanced assignment ----
    cnt_all = np.bincount(dst_all, minlength=N_LATENT)
    latorder = np.argsort(-cnt_all, kind="stable")  # descending count
    lat_core = np.zeros(N_LATENT, np.int64)
    lat_bkt = np.zeros(N_LATENT, np.int64)
    lat_slot = np.zeros(N_LATENT, np.int64)
    slice_max = np.zeros((NCORES, NBKT), np.int64)
    for s in range(NCORES * NBKT):
        b, rr = divmod(s, NCORES)
        c = rr if b % 2 == 0 else NCORES - 1 - rr
        lats = latorder[s * P:(s + 1) * P]
        lat_core[lats] = c
        lat_bkt[lats] = b
        lat_slot[lats] = np.arange(P)
        slice_max[c, b] = max(int(cnt_all[lats].max()), 1)

    # bucket b needs max over cores, rounded to whole superchunks
    supb = [int(-(-slice_max[:, b].max() // NCH)) for b in range(NBKT)]
    nsup = sum(supb)
    ne = nsup * SUP
    bktcols = [s * NCH for s in supb]
    bktcol0 = np.concatenate([[0], np.cumsum(bktcols)])  # column offset/bucket

    # ---- per-edge grid position ----
    order = np.argsort(dst_all, kind="stable")
    sdst = dst_all[order]
    ssrc = src_all[order]
    start = np.searchsorted(sdst, np.arange(N_LATENT))
    jrank = np.arange(E) - start[sdst]
    ecore = lat_core[sdst]
    epos = (bktcol0[lat_bkt[sdst]] + jrank) * P + lat_slot[sdst]

    X9 = np.concatenate(
        [phys_feats[ssrc], phys_pos[ssrc], latent_pos[sdst]],
        axis=1).astype(np.float32)  # [E, 9]

    W1p = np.concatenate([W1[0:3], W1[3:6] - W1[6:9], W1[6:9]], axis=0)
    W1f = np.zeros((10, HID), np.float32)
    W1f[:9] = W1p
    W1f_host = W1f.astype(np.float16)
    W2v = np.ascontiguousarray(
        W2.reshape(2, P, HID).transpose(1, 0, 2).reshape(P, 2 * HID)
    ).astype(np.float16)
    W3p = np.ascontiguousarray(
        W3.reshape(2, P, HID).transpose(1, 0, 2).reshape(P, 2 * HID)
    ).astype(np.float16)
    ones1 = np.ones((1, P), dtype=np.float32)
    b3nz = bool(b3.any())

    in_maps = []
    for c in range(NCORES):
        sel = ecore == c
        Xc = np.zeros((ne, 10), np.float32)
        Xc[epos[sel], :9] = X9[sel]
        xtr = np.ascontiguousarray(Xc.T).astype(np.float16)  # [10, ne]
        # rdiag[l, b, l'] = (l == l') / max(cnt, 1)
        rdiag = np.zeros((P, NBKT, P), np.float32)
        for b in range(NBKT):
            lats = np.where((lat_core == c) & (lat_bkt == b))[0]
            rc = np.zeros(P, np.float32)
            rc[lat_slot[lats]] = 1.0 / np.maximum(cnt_all[lats], 1)
            rdiag[np.arange(P), b, np.arange(P)] = rc
        in_maps.append(dict(
            xtr=xtr, W1f=W1f_host, W2v=W2v, W3p=W3p,
            rdiag=np.ascontiguousarray(
                rdiag.reshape(P, NBKT * P)).astype(np.float16),
            b3r=b3[None, :], ones1=ones1,
        ))

    nc = _build_program(supb, b3nz)
    trace = bool(int(os.environ.get("MAGNO_TRACE", "0")))
    ncores_run = int(os.environ.get("MAGNO_CORES", str(NCORES)))
    res = run_bass_kernel_spmd(nc, in_maps[:ncores_run],
                               core_ids=list(range(ncores_run)), trace=trace)
    last_results = res

    out = np.zeros((N_LATENT, HID), np.float32)
    for c in range(ncores_run):
        oc = np.asarray(res.results[c]["out"])  # [NBKT*P, HID]
        lats = np.where(lat_core == c)[0]
        out[lats] = oc[lat_bkt[lats] * P + lat_slot[lats]]
    return out


# revision 21
# speedup vs baseline: 1.5968x; 1.5968x over previous
"""MAGNO encoder kernel for 8 Trainium2 NeuronCores — v2.

Strategy (dst-sharded, slot-aligned edge grid, fused activations, no gather):
  - Latents are assigned to (core, bucket, slot) on the host, balanced by
    edge count (count-sorted slices of 128, snake-assigned over cores).
    Core c owns NBKT buckets of 128 latent slots; output rows are
    inverse-permuted on the host afterwards.
  - Edges are laid out on the host in a [128 slots x ncols] grid per bucket:
    grid column j is one 128-edge chunk whose slot-p edge has dst == slot p.
    Scatter therefore needs NO per-edge one-hot: chunk tiles are accumulated
    into G[slot, hid] with plain vector adds. Padding slots stream all-zero
    features and contribute exactly 0 (b1 = b2 = 0 per the model spec).
  - Per-edge 10-dim fp16 inputs [f_src, p_src, latent_pos[dst], 0] are packed
    on the host (pure index gather) into a [10, ne] stream: 20 B/edge of HBM
    traffic, no dma_gather, no GpSimd work at all.
  - Algebra: edge_in @ W1 = [f, p, latpos_dst] @ [W1_f; W1_p - W1_rel;
    W1_rel]; W3 is applied after aggregation (it is linear).
  - gelu1(h1 of superchunk k) and gelu2(a2 of superchunk k-2) are fused into
    ONE scalar-engine ACTIVATE over a contiguous [128, 2048] PSUM region,
    amortizing the ~352-cycle ACT fixed overhead. The k-2 pairing keeps the
    scalar engine (the critical resource: 512 gelu evals/edge) bubble-free:
    PSUM region r = k%2 holds [h1(k) | a2(k-2)], and L2(k) overwrites the a2
    section right after ACT(k) reads it.
  - PSUM: exactly two [128, 8, 256] f32 regions = 8 banks.
  - PE program order is software-pipelined (L1(k+2) emitted before L2(k)) so
    the tensor engine never blocks the activation chain.
  - Epilogue per bucket: O = (G @ diag(rcnt))^T-via-matmul @ W3 + b3; the
    1/cnt scaling rides along the transpose matmul using a host-built
    diagonal matrix (counts are index-derived host data).
"""

import os
import numpy as np

import concourse.bass as bass
import concourse.mybir as mybir
import concourse.tile as tile
from concourse import bacc
from concourse.bass_utils import run_bass_kernel_spmd

P = 128
N_PHYS = 100000
N_LATENT = 4096
HID = 256
NCORES = 8
NBKT = 4                      # buckets (slices of 128 latent slots) per core
SUP = 512                     # superchunk edge count (4 chunks of 128)
NCH = SUP // P                # chunks per superchunk = 4
DMACH = 16                    # superchunks per input DMA batch

f32 = mybir.dt.float32
f16 = mybir.dt.float16

last_results = None  # set by kernel(); test harness reads exec_time_ns


def _build_program(supb, b3nz):
    """supb[b]: superchunks in bucket b (same for all cores)."""
    nsup = sum(supb)
    ne = nsup * SUP
    # bucket of superchunk k
    kbkt = []
    for b in range(NBKT):
        kbkt += [b] * supb[b]

    nc = bacc.Bacc("TRN2", target_bir_lowering=False)

    xtr_d = nc.dram_tensor("xtr", [10, ne], f16, kind="ExternalInput")
    W1f_d = nc.dram_tensor("W1f", [10, HID], f16, kind="ExternalInput")
    W2v_d = nc.dram_tensor("W2v", [P, 2 * HID], f16, kind="ExternalInput")
    W3p_d = nc.dram_tensor("W3p", [P, 2 * HID], f16, kind="ExternalInput")
    rdiag_d = nc.dram_tensor("rdiag", [P, NBKT * P], f16, kind="ExternalInput")
    b3_d = nc.dram_tensor("b3r", [1, HID], f32, kind="ExternalInput")
    ones1_d = nc.dram_tensor("ones1", [1, P], f32, kind="ExternalInput")
    out_d = nc.dram_tensor("out", [NBKT * P, HID], f32, kind="ExternalOutput")

    GELU = mybir.ActivationFunctionType.Gelu_apprx_tanh

    with tile.TileContext(nc) as tc:
        with tc.tile_pool(name="const", bufs=1) as cp:

            wu_sb = cp.tile([P, 2, HID], f16, tag="wu_sb")
            nc.vector.memset(wu_sb[:], 1.0)

            def load(shape, dt, src_ap, tag, eng=None):
                t = cp.tile(shape, dt, tag=tag)
                (eng or nc.default_dma_engine).dma_start(out=t[:], in_=src_ap)
                return t

            # main-path weights ride the sync queue (with the xtr stream);
            # epilogue-only tensors go on the default queue in parallel.
            W1f_t = load([10, HID], f16, W1f_d[:], "W1f", nc.sync)
            W2v_t = load([P, 2, HID], f16, W2v_d[:], "W2v", nc.sync)
            W3p_t = load([P, 2, HID], f16, W3p_d[:], "W3p")
            rdiag_t = load([P, NBKT, P], f16, rdiag_d[:], "rdiag")
            if b3nz:
                b3_t = load([1, HID], f32, b3_d[:], "b3")
                ones1_t = load([1, P], f32, ones1_d[:], "ones1")

            # persistent SBUF accumulators per bucket, zeroed
            Gsb = [cp.tile([P, HID], f32, tag=f"Gsb{b}", name=f"Gsb{b}")
                   for b in range(NBKT)]
            for b in range(NBKT):
                nc.gpsimd.memset(Gsb[b][:], 0.0)

            with tc.tile_pool(name="psR", bufs=1, space="PSUM") as psR, \
                 tc.tile_pool(name="xin", bufs=3) as xp, \
                 tc.tile_pool(name="act", bufs=3) as ap_, \
                 tc.tile_pool(name="sc", bufs=3) as sp:

                # PSUM ping-pong regions: [h1 (2x512) | a2 (4x256)] = 4 banks
                REG = [psR.tile([P, 8, 256], f32, tag=f"REG{x}",
                                name=f"REG{x}") for x in range(2)]

                # PE warm-up: ~4us of back-to-back matmuls flips the HAM
                # clock gate from 4/8 (1.2 GHz) to 8/8 (2.4 GHz); the loop's
                # filler matmuls then keep it warm. Targets the h1 area that
                # L1(0) overwrites, so no extra PSUM and no pool barrier.
                for _ in range(9):
                    nc.tensor.matmul(out=REG[0][:, 0:2, :],
                                     lhsT=wu_sb[:, 0, 0:P], rhs=wu_sb[:],
                                     start=True, stop=True,
                                     skip_group_check=True)

                xt_tiles = {}

                def ensure_xt(k):
                    """Emit the DMA for the batch containing superchunk k."""
                    nb = k // DMACH
                    if nb in xt_tiles or k >= nsup:
                        return
                    nsw = min(DMACH, nsup - nb * DMACH)
                    t = xp.tile([10, DMACH, SUP], f16, tag="xt")
                    nc.sync.dma_start(
                        out=t[:, :nsw, :],
                        in_=xtr_d[:, nb * DMACH * SUP:
                                  (nb * DMACH + nsw) * SUP])
                    xt_tiles[nb] = t
                    xt_tiles.pop(nb - 2, None)

                def emit_L1(k):
                    if k >= nsup:
                        return
                    ensure_xt(k)
                    r = k % 2
                    xt = xt_tiles[k // DMACH]
                    for m in range(2):
                        nc.tensor.matmul(
                            out=REG[r][:, 2 * m:2 * m + 2, :],
                            lhsT=W1f_t[:, m * P:(m + 1) * P],
                            rhs=xt[:, k % DMACH, :],
                            start=True, stop=True,
                            skip_group_check=True)

                def emit_DVE_scatter(ao, ksrc):
                    """Fold a2h(ksrc) (= ao[:, 4:8, :]) into Gsb[bucket]."""
                    b = kbkt[ksrc]
                    s1 = sp.tile([P, HID], f16, tag="s1")
                    nc.vector.tensor_tensor(out=s1[:], in0=ao[:, 4, :],
                                            in1=ao[:, 5, :],
                                            op=mybir.AluOpType.add)
                    s2 = sp.tile([P, HID], f16, tag="s2")
                    nc.vector.tensor_tensor(out=s2[:], in0=ao[:, 6, :],
                                            in1=ao[:, 7, :],
                                            op=mybir.AluOpType.add)
                    s3 = sp.tile([P, HID], f16, tag="s3")
                    nc.vector.tensor_tensor(out=s3[:], in0=s1[:], in1=s2[:],
                                            op=mybir.AluOpType.add)
                    s4 = sp.tile([P, HID], f32, tag="s4")
                    nc.vector.tensor_copy(out=s4[:], in_=s3[:])
                    nc.vector.tensor_tensor(out=Gsb[b][:], in0=Gsb[b][:],
                                            in1=s4[:],
                                            op=mybir.AluOpType.add)

                ndummy = int(os.environ.get("MAGNO_DUMMY", "2"))
                ndldw = int(os.environ.get("MAGNO_DLDW", "0"))
                emit_L1(0)
                emit_L1(1)
                for k in range(nsup):
                    r = k % 2
                    # fused ACT(k): gelu over [h1(k) | a2(k-2)]
                    ao = ap_.tile([P, 8, 256], f16, tag="ao")
                    if k >= 2:
                        nc.scalar.activation(out=ao[:], in_=REG[r][:],
                                             func=GELU)
                    else:
                        nc.scalar.activation(out=ao[:, 0:4, :],
                                             in_=REG[r][:, 0:4, :], func=GELU)
                    # PE filler: dummy matmuls into the h1 area that L1
                    # overwrites right after; enough PE duty to keep the HAM
                    # clock gate at 8/8 (2.4 GHz).
                    for d in range(ndummy):
                        nc.tensor.matmul(out=REG[r][:, 0:2, :],
                                         lhsT=wu_sb[:, 0, 0:P],
                                         rhs=wu_sb[:],
                                         start=True, stop=True,
                                         skip_group_check=True)
                    # PE: L1(k+2) first, then L2(k) (keeps ACT chain fed)
                    emit_L1(k + 2)
                    ensure_xt(k + 2 + DMACH)
                    for j in range(NCH):
                        q = j // 2
                        o = (j % 2) * P
                        for m in range(2):
                            nc.tensor.matmul(
                                out=REG[r][:, 4 + j, :],
                                lhsT=ao[:, 2 * m + q, o:o + P],
                                rhs=W2v_t[:, m, :],
                                start=(m == 0), stop=(m == 1),
                                skip_group_check=True)
                    # PE filler (off-chain): dummy LDWEIGHTS after the real
                    # block keep the HAM clock gate at 8/8 (2.4 GHz) without
                    # touching PSUM or sitting on the ACT dependency chain.
                    for _ in range(ndldw):
                        nc.tensor.ldweights(weights=W2v_t[:, 0, 0:P])
                    # DVE: scatter a2h(k-2)
                    if k >= 2:
                        emit_DVE_scatter(ao, k - 2)

                # tails: a2(nsup-2), a2(nsup-1) still in PSUM
                for ktail in (nsup - 2, nsup - 1):
                    r = ktail % 2
                    ao = ap_.tile([P, 8, 256], f16, tag="ao")
                    nc.scalar.activation(out=ao[:, 4:8, :],
                                         in_=REG[r][:, 4:8, :], func=GELU)
                    emit_DVE_scatter(ao, ktail)

            # ---- epilogue: O = (Gsb @ diag(rcnt))^T @ W3 (+ b3) ----
            with tc.tile_pool(name="ep", bufs=2) as ep, \
                 tc.tile_pool(name="psE", bufs=2, space="PSUM") as psE:
                if b3nz:
                    b3_ps = psE.tile([P, HID], f32, tag="b3bc")
                    nc.tensor.matmul(out=b3_ps[:], lhsT=ones1_t[:], rhs=b3_t[:],
                                     start=True, stop=True)
                    b3bc_t = ep.tile([P, HID], f32, tag="b3bc")
                    nc.vector.tensor_copy(out=b3bc_t[:], in_=b3_ps[:])
                for b in range(NBKT):
                    # gt[h, l] = Gsb[l, h] * rcnt[l] via matmul with diag
                    gsh_t = ep.tile([P, HID], f16, tag="gsh")
                    nc.vector.tensor_copy(out=gsh_t[:], in_=Gsb[b][:])
                    gth_t = ep.tile([P, 2, P], f16, tag="gth")
                    for q in range(2):
                        gt_ps = psE.tile([P, P], f32, tag="gt")
                        nc.tensor.matmul(out=gt_ps[:],
                                         lhsT=gsh_t[:, q * P:(q + 1) * P],
                                         rhs=rdiag_t[:, b, :],
                                         start=True, stop=True)
                        nc.vector.tensor_copy(out=gth_t[:, q, :], in_=gt_ps[:])
                    o_ps = psE.tile([P, HID], f32, tag="o")
                    nc.tensor.matmul(out=o_ps[:], lhsT=gth_t[:, 0, :],
                                     rhs=W3p_t[:, 0, :], start=True, stop=False)
                    nc.tensor.matmul(out=o_ps[:], lhsT=gth_t[:, 1, :],
                                     rhs=W3p_t[:, 1, :], start=False, stop=True)
                    o_t = ep.tile([P, HID], f32, tag="osb")
                    if b3nz:
                        nc.vector.tensor_tensor(out=o_t[:], in0=o_ps[:],
                                                in1=b3bc_t[:],
                                                op=mybir.AluOpType.add)
                    else:
                        nc.vector.tensor_copy(out=o_t[:], in_=o_ps[:])
                    nc.default_dma_engine.dma_start(
                        out=out_d[b * P:(b + 1) * P, :], in_=o_t[:])

    nc.finalize()
    return nc


def kernel(phys_feats, phys_pos, latent_pos, edge_src, edge_dst,
           W1, b1, W2, b2, W3, b3):
    global last_results
    phys_feats = np.asarray(phys_feats, dtype=np.float32)
    phys_pos = np.asarray(phys_pos, dtype=np.float32)
    latent_pos = np.asarray(latent_pos, dtype=np.float32)
    W1 = np.asarray(W1, dtype=np.float32)
    W2 = np.asarray(W2, dtype=np.float32)
    W3 = np.asarray(W3, dtype=np.float32)
    b1 = np.asarray(b1, dtype=np.float32)
    b2 = np.asarray(b2, dtype=np.float32)
    b3 = np.asarray(b3, dtype=np.float32)
    src_all = np.asarray(edge_src).reshape(-1).astype(np.int64)
    dst_all = np.asarray(edge_dst).reshape(-1).astype(np.int64)
    E = src_all.shape[0]
    assert not b1.any() and not b2.any(), "zero b1/b2 assumed (spec fill=zeros)"

    # ---- latent -> (core, bucket, slot) balanced assignment ----
    cnt_all = np.bincount(dst_all, minlength=N_LATENT)
    latorder = np.argsort(-cnt_all, kind="stable")  # descending count
    lat_core = np.zeros(N_LATENT, np.int64)
    lat_bkt = np.zeros(N_LATENT, np.int64)
    lat_slot = np.zeros(N_LATENT, np.int64)
    slice_max = np.zeros((NCORES, NBKT), np.int64)
    for s in range(NCORES * NBKT):
        b, rr = divmod(s, NCORES)
        c = rr if b % 2 == 0 else NCORES - 1 - rr
        lats = latorder[s * P:(s + 1) * P]
        lat_core[lats] = c
        lat_bkt[lats] = b
        lat_slot[lats] = np.arange(P)
        slice_max[c, b] = max(int(cnt_all[lats].max()), 1)

    # bucket b needs max over cores, rounded to whole superchunks
    supb = [int(-(-slice_max[:, b].max() // NCH)) for b in range(NBKT)]
    nsup = sum(supb)
    ne = nsup * SUP
    bktcols = [s * NCH for s in supb]
    bktcol0 = np.concatenate([[0], np.cumsum(bktcols)])  # column offset/bucket

    # ---- per-edge grid position ----
    order = np.argsort(dst_all, kind="stable")
    sdst = dst_all[order]
    ssrc = src_all[order]
    start = np.searchsorted(sdst, np.arange(N_LATENT))
    jrank = np.arange(E) - start[sdst]
    ecore = lat_core[sdst]
    epos = (bktcol0[lat_bkt[sdst]] + jrank) * P + lat_slot[sdst]

    X9 = np.concatenate(
        [phys_feats[ssrc], phys_pos[ssrc], latent_pos[sdst]],
        axis=1).astype(np.float32)  # [E, 9]

    W1p = np.concatenate([W1[0:3], W1[3:6] - W1[6:9], W1[6:9]], axis=0)
    W1f = np.zeros((10, HID), np.float32)
    W1f[:9] = W1p
    W1f_host = W1f.astype(np.float16)
    W2v = np.ascontiguousarray(
        W2.reshape(2, P, HID).transpose(1, 0, 2).reshape(P, 2 * HID)
    ).astype(np.float16)
    W3p = np.ascontiguousarray(
        W3.reshape(2, P, HID).transpose(1, 0, 2).reshape(P, 2 * HID)
    ).astype(np.float16)
    ones1 = np.ones((1, P), dtype=np.float32)
    b3nz = bool(b3.any())

    in_maps = []
    for c in range(NCORES):
        sel = ecore == c
        Xc = np.zeros((ne, 10), np.float32)
        Xc[epos[sel], :9] = X9[sel]
        xtr = np.ascontiguousarray(Xc.T).astype(np.float16)  # [10, ne]
        # rdiag[l, b, l'] = (l == l') / max(cnt, 1)
        rdiag = np.zeros((P, NBKT, P), np.float32)
        for b in range(NBKT):
            lats = np.where((lat_core == c) & (lat_bkt == b))[0]
            rc = np.zeros(P, np.float32)
            rc[lat_slot[lats]] = 1.0 / np.maximum(cnt_all[lats], 1)
            rdiag[np.arange(P), b, np.arange(P)] = rc
        in_maps.append(dict(
            xtr=xtr, W1f=W1f_host, W2v=W2v, W3p=W3p,
            rdiag=np.ascontiguousarray(
                rdiag.reshape(P, NBKT * P)).astype(np.float16),
            b3r=b3[None, :], ones1=ones1,
        ))

    nc = _build_program(supb, b3nz)
    trace = bool(int(os.environ.get("MAGNO_TRACE", "0")))
    ncores_run = int(os.environ.get("MAGNO_CORES", str(NCORES)))
    res = run_bass_kernel_spmd(nc, in_maps[:ncores_run],
                               core_ids=list(range(ncores_run)), trace=trace)
    last_results = res

    out = np.zeros((N_LATENT, HID), np.float32)
    for c in range(ncores_run):
        oc = np.asarray(res.results[c]["out"])  # [NBKT*P, HID]
        lats = np.where(lat_core == c)[0]
        out[lats] = oc[lat_bkt[lats] * P + lat_slot[lats]]
    return out


# revision 23
# speedup vs baseline: 1.9042x; 1.1925x over previous
"""MAGNO encoder kernel for 8 Trainium2 NeuronCores — v2.

Strategy (dst-sharded, slot-aligned edge grid, fused activations, no gather):
  - Latents are assigned to (core, bucket, slot) on the host, balanced by
    edge count (count-sorted slices of 128, snake-assigned over cores).
    Core c owns NBKT buckets of 128 latent slots; output rows are
    inverse-permuted on the host afterwards.
  - Edges are laid out on the host in a [128 slots x ncols] grid per bucket:
    grid column j is one 128-edge chunk whose slot-p edge has dst == slot p.
    Scatter therefore needs NO per-edge one-hot: chunk tiles are accumulated
    into G[slot, hid] with plain vector adds. Padding slots stream all-zero
    features and contribute exactly 0 (b1 = b2 = 0 per the model spec).
  - Per-edge 10-dim fp16 inputs [f_src, p_src, latent_pos[dst], 0] are packed
    on the host (pure index gather) into a [10, ne] stream: 20 B/edge of HBM
    traffic, no dma_gather, no GpSimd work at all.
  - Algebra: edge_in @ W1 = [f, p, latpos_dst] @ [W1_f; W1_p - W1_rel;
    W1_rel]; W3 is applied after aggregation (it is linear).
  - gelu1(h1 of superchunk k) and gelu2(a2 of superchunk k-2) are fused into
    ONE scalar-engine ACTIVATE over a contiguous [128, 2048] PSUM region,
    amortizing the ~352-cycle ACT fixed overhead. The k-2 pairing keeps the
    scalar engine (the critical resource: 512 gelu evals/edge) bubble-free:
    PSUM region r = k%2 holds [h1(k) | a2(k-2)], and L2(k) overwrites the a2
    section right after ACT(k) reads it.
  - PSUM: exactly two [128, 8, 256] f32 regions = 8 banks.
  - PE program order is software-pipelined (L1(k+2) emitted before L2(k)) so
    the tensor engine never blocks the activation chain.
  - Epilogue per bucket: O = (G @ diag(rcnt))^T-via-matmul @ W3 + b3; the
    1/cnt scaling rides along the transpose matmul using a host-built
    diagonal matrix (counts are index-derived host data).
"""

import os
import numpy as np

import concourse.bass as bass
import concourse.mybir as mybir
import concourse.tile as tile
from concourse import bacc
from concourse.bass_utils import run_bass_kernel_spmd

P = 128
N_PHYS = 100000
N_LATENT = 4096
HID = 256
NCORES = 8
NBKT = 4                      # buckets (slices of 128 latent slots) per core
SUP = 512                     # superchunk edge count (4 chunks of 128)
NCH = SUP // P                # chunks per superchunk = 4
DMACH = 16                    # superchunks per input DMA batch

f32 = mybir.dt.float32
f16 = mybir.dt.float16

last_results = None  # set by kernel(); test harness reads exec_time_ns


def _build_program(supb, b3nz):
    """supb[b]: superchunks in bucket b (same for all cores)."""
    nsup = sum(supb)
    ne = nsup * SUP
    # bucket of superchunk k
    kbkt = []
    for b in range(NBKT):
        kbkt += [b] * supb[b]

    nc = bacc.Bacc("TRN2", target_bir_lowering=False)

    xtr_d = nc.dram_tensor("xtr", [10, ne], f16, kind="ExternalInput")
    W1f_d = nc.dram_tensor("W1f", [10, HID], f16, kind="ExternalInput")
    W2v_d = nc.dram_tensor("W2v", [P, 2 * HID], f16, kind="ExternalInput")
    W3p_d = nc.dram_tensor("W3p", [P, 2 * HID], f16, kind="ExternalInput")
    rdiag_d = nc.dram_tensor("rdiag", [P, NBKT * P], f16, kind="ExternalInput")
    b3_d = nc.dram_tensor("b3r", [1, HID], f32, kind="ExternalInput")
    ones1_d = nc.dram_tensor("ones1", [1, P], f32, kind="ExternalInput")
    out_d = nc.dram_tensor("out", [NBKT * P, HID], f32, kind="ExternalOutput")

    GELU = mybir.ActivationFunctionType.Gelu_apprx_tanh

    with tile.TileContext(nc) as tc:
        with tc.tile_pool(name="const", bufs=1) as cp:

            wu_sb = cp.tile([P, 2, HID], f16, tag="wu_sb")
            nc.vector.memset(wu_sb[:], 1.0)

            def load(shape, dt, src_ap, tag, eng=None):
                t = cp.tile(shape, dt, tag=tag)
                (eng or nc.default_dma_engine).dma_start(out=t[:], in_=src_ap)
                return t

            # main-path weights ride the sync queue (with the xtr stream);
            # epilogue-only tensors go on the default queue in parallel.
            W1f_t = load([10, HID], f16, W1f_d[:], "W1f", nc.sync)
            W2v_t = load([P, 2, HID], f16, W2v_d[:], "W2v", nc.sync)
            W3p_t = load([P, 2, HID], f16, W3p_d[:], "W3p")
            rdiag_t = load([P, NBKT, P], f16, rdiag_d[:], "rdiag")
            if b3nz:
                b3_t = load([1, HID], f32, b3_d[:], "b3")
                ones1_t = load([1, P], f32, ones1_d[:], "ones1")

            # persistent SBUF accumulators per bucket, zeroed
            Gsb = [cp.tile([P, HID], f32, tag=f"Gsb{b}", name=f"Gsb{b}")
                   for b in range(NBKT)]
            for b in range(NBKT):
                nc.gpsimd.memset(Gsb[b][:], 0.0)

            with tc.tile_pool(name="psR", bufs=1, space="PSUM") as psR, \
                 tc.tile_pool(name="xin", bufs=2) as xp, \
                 tc.tile_pool(name="act", bufs=3) as ap_, \
                 tc.tile_pool(name="sc", bufs=3) as sp:

                # PSUM ping-pong regions: [h1 (2x512) | a2 (4x256)] = 4 banks
                REG = [psR.tile([P, 8, 256], f32, tag=f"REG{x}",
                                name=f"REG{x}") for x in range(2)]

                # PE warm-up: ~4us of back-to-back matmuls flips the HAM
                # clock gate from 4/8 (1.2 GHz) to 8/8 (2.4 GHz); the loop's
                # filler matmuls then keep it warm. Targets the h1 area that
                # L1(0) overwrites, so no extra PSUM and no pool barrier.
                for _ in range(9):
                    nc.tensor.matmul(out=REG[0][:, 0:2, :],
                                     lhsT=wu_sb[:, 0, 0:P], rhs=wu_sb[:],
                                     start=True, stop=True,
                                     skip_group_check=True)

                xt_tiles = {}

                def ensure_xt(k):
                    """Emit the DMA for the batch containing superchunk k."""
                    nb = k // DMACH
                    if nb in xt_tiles or k >= nsup:
                        return
                    nsw = min(DMACH, nsup - nb * DMACH)
                    t = xp.tile([10, DMACH, SUP], f16, tag="xt")
                    nc.sync.dma_start(
                        out=t[:, :nsw, :],
                        in_=xtr_d[:, nb * DMACH * SUP:
                                  (nb * DMACH + nsw) * SUP])
                    xt_tiles[nb] = t
                    xt_tiles.pop(nb - 2, None)

                def emit_L1(k):
                    if k >= nsup:
                        return
                    ensure_xt(k)
                    r = k % 2
                    xt = xt_tiles[k // DMACH]
                    for m in range(2):
                        nc.tensor.matmul(
                            out=REG[r][:, 2 * m:2 * m + 2, :],
                            lhsT=W1f_t[:, m * P:(m + 1) * P],
                            rhs=xt[:, k % DMACH, :],
                            start=True, stop=True,
                            skip_group_check=True)

                def emit_DVE_scatter(ao, ksrc):
                    """Fold a2h(ksrc) (= ao[:, 4:8, :]) into Gsb[bucket]."""
                    b = kbkt[ksrc]
                    s1 = sp.tile([P, HID], f16, tag="s1")
                    nc.vector.tensor_tensor(out=s1[:], in0=ao[:, 4, :],
                                            in1=ao[:, 5, :],
                                            op=mybir.AluOpType.add)
                    s2 = sp.tile([P, HID], f16, tag="s2")
                    nc.vector.tensor_tensor(out=s2[:], in0=ao[:, 6, :],
                                            in1=ao[:, 7, :],
                                            op=mybir.AluOpType.add)
                    s3 = sp.tile([P, HID], f16, tag="s3")
                    nc.vector.tensor_tensor(out=s3[:], in0=s1[:], in1=s2[:],
                                            op=mybir.AluOpType.add)
                    s4 = sp.tile([P, HID], f32, tag="s4")
                    nc.vector.tensor_copy(out=s4[:], in_=s3[:])
                    nc.vector.tensor_tensor(out=Gsb[b][:], in0=Gsb[b][:],
                                            in1=s4[:],
                                            op=mybir.AluOpType.add)

                ndummy = int(os.environ.get("MAGNO_DUMMY", "2"))
                ndldw = int(os.environ.get("MAGNO_DLDW", "0"))
                emit_L1(0)
                emit_L1(1)
                for k in range(nsup):
                    r = k % 2
                    # fused ACT(k): gelu over [h1(k) | a2(k-2)]
                    ao = ap_.tile([P, 8, 256], f16, tag="ao")
                    if k >= 2:
                        nc.scalar.activation(out=ao[:], in_=REG[r][:],
                                             func=GELU)
                    else:
                        nc.scalar.activation(out=ao[:, 0:4, :],
                                             in_=REG[r][:, 0:4, :], func=GELU)
                    # PE filler: dummy matmuls into the h1 area that L1
                    # overwrites right after; enough PE duty to keep the HAM
                    # clock gate at 8/8 (2.4 GHz).
                    for d in range(ndummy):
                        nc.tensor.matmul(out=REG[r][:, 0:2, :],
                                         lhsT=wu_sb[:, 0, 0:P],
                                         rhs=wu_sb[:],
                                         start=True, stop=True,
                                         skip_group_check=True)
                    # PE: L1(k+2) first, then L2(k) (keeps ACT chain fed)
                    emit_L1(k + 2)
                    for j in range(NCH):
                        q = j // 2
                        o = (j % 2) * P
                        for m in range(2):
                            nc.tensor.matmul(
                                out=REG[r][:, 4 + j, :],
                                lhsT=ao[:, 2 * m + q, o:o + P],
                                rhs=W2v_t[:, m, :],
                                start=(m == 0), stop=(m == 1),
                                skip_group_check=True)
                    # PE filler (off-chain): dummy LDWEIGHTS after the real
                    # block keep the HAM clock gate at 8/8 (2.4 GHz) without
                    # touching PSUM or sitting on the ACT dependency chain.
                    for _ in range(ndldw):
                        nc.tensor.ldweights(weights=W2v_t[:, 0, 0:P])
                    # DVE: scatter a2h(k-2)
                    if k >= 2:
                        emit_DVE_scatter(ao, k - 2)

                # tails: a2(nsup-2), a2(nsup-1) still in PSUM
                for ktail in (nsup - 2, nsup - 1):
                    r = ktail % 2
                    ao = ap_.tile([P, 8, 256], f16, tag="ao")
                    nc.scalar.activation(out=ao[:, 4:8, :],
                                         in_=REG[r][:, 4:8, :], func=GELU)
                    emit_DVE_scatter(ao, ktail)

            # ---- epilogue: O = (Gsb @ diag(rcnt))^T @ W3 (+ b3) ----
            with tc.tile_pool(name="ep", bufs=2) as ep, \
                 tc.tile_pool(name="psE", bufs=2, space="PSUM") as psE:
                if b3nz:
                    b3_ps = psE.tile([P, HID], f32, tag="b3bc")
                    nc.tensor.matmul(out=b3_ps[:], lhsT=ones1_t[:], rhs=b3_t[:],
                                     start=True, stop=True)
                    b3bc_t = ep.tile([P, HID], f32, tag="b3bc")
                    nc.vector.tensor_copy(out=b3bc_t[:], in_=b3_ps[:])
                for b in range(NBKT):
                    # gt[h, l] = Gsb[l, h] * rcnt[l] via matmul with diag
                    gsh_t = ep.tile([P, HID], f16, tag="gsh")
                    nc.vector.tensor_copy(out=gsh_t[:], in_=Gsb[b][:])
                    gth_t = ep.tile([P, 2, P], f16, tag="gth")
                    for q in range(2):
                        gt_ps = psE.tile([P, P], f32, tag="gt")
                        nc.tensor.matmul(out=gt_ps[:],
                                         lhsT=gsh_t[:, q * P:(q + 1) * P],
                                         rhs=rdiag_t[:, b, :],
                                         start=True, stop=True)
                        nc.vector.tensor_copy(out=gth_t[:, q, :], in_=gt_ps[:])
                    o_ps = psE.tile([P, HID], f32, tag="o")
                    nc.tensor.matmul(out=o_ps[:], lhsT=gth_t[:, 0, :],
                                     rhs=W3p_t[:, 0, :], start=True, stop=False)
                    nc.tensor.matmul(out=o_ps[:], lhsT=gth_t[:, 1, :],
                                     rhs=W3p_t[:, 1, :], start=False, stop=True)
                    o_t = ep.tile([P, HID], f32, tag="osb")
                    if b3nz:
                        nc.vector.tensor_tensor(out=o_t[:], in0=o_ps[:],
                                                in1=b3bc_t[:],
                                                op=mybir.AluOpType.add)
                    else:
                        nc.vector.tensor_copy(out=o_t[:], in_=o_ps[:])
                    nc.default_dma_engine.dma_start(
                        out=out_d[b * P:(b + 1) * P, :], in_=o_t[:])

    nc.finalize()
    return nc


def kernel(phys_feats, phys_pos, latent_pos, edge_src, edge_dst,
           W1, b1, W2, b2, W3, b3):
    global last_results
    phys_feats = np.asarray(phys_feats, dtype=np.float32)
    phys_pos = np.asarray(phys_pos, dtype=np.float32)
    latent_pos = np.asarray(latent_pos, dtype=np.float32)
    W1 = np.asarray(W1, dtype=np.float32)
    W2 = np.asarray(W2, dtype=np.float32)
    W3 = np.asarray(W3, dtype=np.float32)
    b1 = np.asarray(b1, dtype=np.float32)
    b2 = np.asarray(b2, dtype=np.float32)
    b3 = np.asarray(b3, dtype=np.float32)
    src_all = np.asarray(edge_src).reshape(-1).astype(np.int64)
    dst_all = np.asarray(edge_dst).reshape(-1).astype(np.int64)
    E = src_all.shape[0]
    assert not b1.any() and not b2.any(), "zero b1/b2 assumed (spec fill=zeros)"

    # ---- latent -> (core, bucket, slot) balanced assignment ----
    cnt_all = np.bincount(dst_all, minlength=N_LATENT)
    latorder = np.argsort(-cnt_all, kind="stable")  # descending count
    lat_core = np.zeros(N_LATENT, np.int64)
    lat_bkt = np.zeros(N_LATENT, np.int64)
    lat_slot = np.zeros(N_LATENT, np.int64)
    slice_max = np.zeros((NCORES, NBKT), np.int64)
    for s in range(NCORES * NBKT):
        b, rr = divmod(s, NCORES)
        c = rr if b % 2 == 0 else NCORES - 1 - rr
        lats = latorder[s * P:(s + 1) * P]
        lat_core[lats] = c
        lat_bkt[lats] = b
        lat_slot[lats] = np.arange(P)
        slice_max[c, b] = max(int(cnt_all[lats].max()), 1)

    # bucket b needs max over cores, rounded to whole superchunks
    supb = [int(-(-slice_max[:, b].max() // NCH)) for b in range(NBKT)]
    nsup = sum(supb)
    ne = nsup * SUP
    bktcols = [s * NCH for s in supb]
    bktcol0 = np.concatenate([[0], np.cumsum(bktcols)])  # column offset/bucket

    # ---- per-edge grid position ----
    order = np.argsort(dst_all, kind="stable")
    sdst = dst_all[order]
    ssrc = src_all[order]
    start = np.searchsorted(sdst, np.arange(N_LATENT))
    jrank = np.arange(E) - start[sdst]
    ecore = lat_core[sdst]
    epos = (bktcol0[lat_bkt[sdst]] + jrank) * P + lat_slot[sdst]

    X9 = np.concatenate(
        [phys_feats[ssrc], phys_pos[ssrc], latent_pos[sdst]],
        axis=1).astype(np.float32)  # [E, 9]

    W1p = np.concatenate([W1[0:3], W1[3:6] - W1[6:9], W1[6:9]], axis=0)
    W1f = np.zeros((10, HID), np.float32)
    W1f[:9] = W1p
    W1f_host = W1f.astype(np.float16)
    W2v = np.ascontiguousarray(
        W2.reshape(2, P, HID).transpose(1, 0, 2).reshape(P, 2 * HID)
    ).astype(np.float16)
    W3p = np.ascontiguousarray(
        W3.reshape(2, P, HID).transpose(1, 0, 2).reshape(P, 2 * HID)
    ).astype(np.float16)
    ones1 = np.ones((1, P), dtype=np.float32)
    b3nz = bool(b3.any())

    in_maps = []
    for c in range(NCORES):
        sel = ecore == c
        Xc = np.zeros((ne, 10), np.float32)
        Xc[epos[sel], :9] = X9[sel]
        xtr = np.ascontiguousarray(Xc.T).astype(np.float16)  # [10, ne]
        # rdiag[l, b, l'] = (l == l') / max(cnt, 1)
        rdiag = np.zeros((P, NBKT, P), np.float32)
        for b in range(NBKT):
            lats = np.where((lat_core == c) & (lat_bkt == b))[0]
            rc = np.zeros(P, np.float32)
            rc[lat_slot[lats]] = 1.0 / np.maximum(cnt_all[lats], 1)
            rdiag[np.arange(P), b, np.arange(P)] = rc
        in_maps.append(dict(
            xtr=xtr, W1f=W1f_host, W2v=W2v, W3p=W3p,
            rdiag=np.ascontiguousarray(
                rdiag.reshape(P, NBKT * P)).astype(np.float16),
            b3r=b3[None, :], ones1=ones1,
        ))

    nc = _build_program(supb, b3nz)
    trace = bool(int(os.environ.get("MAGNO_TRACE", "0")))
    ncores_run = int(os.environ.get("MAGNO_CORES", str(NCORES)))
    res = run_bass_kernel_spmd(nc, in_maps[:ncores_run],
                               core_ids=list(range(ncores_run)), trace=trace)
    last_results = res

    out = np.zeros((N_LATENT, HID), np.float32)
    for c in range(ncores_run):
        oc = np.asarray(res.results[c]["out"])  # [NBKT*P, HID]
        lats = np.where(lat_core == c)[0]
        out[lats] = oc[lat_bkt[lats] * P + lat_slot[lats]]
    return out


# revision 30
# speedup vs baseline: 1.9043x; 1.0000x over previous
"""MAGNO encoder kernel for 8 Trainium2 NeuronCores — v2.

Strategy (dst-sharded, slot-aligned edge grid, fused activations, no gather):
  - Latents are assigned to (core, bucket, slot) on the host, balanced by
    edge count (count-sorted slices of 128, snake-assigned over cores).
    Core c owns NBKT buckets of 128 latent slots; output rows are
    inverse-permuted on the host afterwards.
  - Edges are laid out on the host in a [128 slots x ncols] grid per bucket:
    grid column j is one 128-edge chunk whose slot-p edge has dst == slot p.
    Scatter therefore needs NO per-edge one-hot: chunk tiles are accumulated
    into G[slot, hid] with plain vector adds. Padding slots stream all-zero
    features and contribute exactly 0 (b1 = b2 = 0 per the model spec).
  - Per-edge 10-dim fp16 inputs [f_src, p_src, latent_pos[dst], 0] are packed
    on the host (pure index gather) into a [10, ne] stream: 20 B/edge of HBM
    traffic, no dma_gather, no GpSimd work at all.
  - Algebra: edge_in @ W1 = [f, p, latpos_dst] @ [W1_f; W1_p - W1_rel;
    W1_rel]; W3 is applied after aggregation (it is linear).
  - gelu1(h1 of superchunk k) and gelu2(a2 of superchunk k-2) are fused into
    ONE scalar-engine ACTIVATE over a contiguous [128, 2048] PSUM region,
    amortizing the ~352-cycle ACT fixed overhead. The k-2 pairing keeps the
    scalar engine (the critical resource: 512 gelu evals/edge) bubble-free:
    PSUM region r = k%2 holds [h1(k) | a2(k-2)], and L2(k) overwrites the a2
    section right after ACT(k) reads it.
  - PSUM: exactly two [128, 8, 256] f32 regions = 8 banks.
  - PE program order is software-pipelined (L1(k+2) emitted before L2(k)) so
    the tensor engine never blocks the activation chain.
  - Epilogue per bucket: O = (G @ diag(rcnt))^T-via-matmul @ W3 + b3; the
    1/cnt scaling rides along the transpose matmul using a host-built
    diagonal matrix (counts are index-derived host data).
"""

import os
import numpy as np

import concourse.bass as bass
import concourse.mybir as mybir
import concourse.tile as tile
from concourse import bacc
from concourse.bass_utils import run_bass_kernel_spmd

P = 128
N_PHYS = 100000
N_LATENT = 4096
HID = 256
NCORES = 8
NBKT = 4                      # buckets (slices of 128 latent slots) per core
SUP = 512                     # superchunk edge count (4 chunks of 128)
NCH = SUP // P                # chunks per superchunk = 4
DMACH = 16                    # superchunks per input DMA batch

f32 = mybir.dt.float32
f16 = mybir.dt.float16

last_results = None  # set by kernel(); test harness reads exec_time_ns


def _build_program(supb, b3nz):
    """supb[b]: superchunks in bucket b (same for all cores)."""
    nsup = sum(supb)
    ne = nsup * SUP
    # bucket of superchunk k
    kbkt = []
    for b in range(NBKT):
        kbkt += [b] * supb[b]

    nc = bacc.Bacc("TRN2", target_bir_lowering=False)

    xtr_d = nc.dram_tensor("xtr", [10, ne], f16, kind="ExternalInput")
    W1f_d = nc.dram_tensor("W1f", [10, HID], f16, kind="ExternalInput")
    W2v_d = nc.dram_tensor("W2v", [P, 2 * HID], f16, kind="ExternalInput")
    W3p_d = nc.dram_tensor("W3p", [P, 2 * HID], f16, kind="ExternalInput")
    rdiag_d = nc.dram_tensor("rdiag", [P, NBKT * P], f16, kind="ExternalInput")
    b3_d = nc.dram_tensor("b3r", [1, HID], f32, kind="ExternalInput")
    ones1_d = nc.dram_tensor("ones1", [1, P], f32, kind="ExternalInput")
    out_d = nc.dram_tensor("out", [NBKT * P, HID], f32, kind="ExternalOutput")

    GELU = mybir.ActivationFunctionType.Gelu_apprx_tanh

    with tile.TileContext(nc) as tc:
        with tc.tile_pool(name="const", bufs=1) as cp:

            wu_sb = cp.tile([P, 2, HID], f16, tag="wu_sb")
            nc.vector.memset(wu_sb[:], 1.0)

            def load(shape, dt, src_ap, tag, eng=None):
                t = cp.tile(shape, dt, tag=tag)
                (eng or nc.default_dma_engine).dma_start(out=t[:], in_=src_ap)
                return t

            # main-path weights ride the sync queue (with the xtr stream);
            # epilogue-only tensors go on the default queue in parallel.
            W1f_t = load([10, HID], f16, W1f_d[:], "W1f", nc.sync)
            W2v_t = load([P, 2, HID], f16, W2v_d[:], "W2v", nc.sync)
            W3p_t = load([P, 2, HID], f16, W3p_d[:], "W3p")
            rdiag_t = load([P, NBKT, P], f16, rdiag_d[:], "rdiag")
            if b3nz:
                b3_t = load([1, HID], f32, b3_d[:], "b3")
                ones1_t = load([1, P], f32, ones1_d[:], "ones1")

            # persistent SBUF accumulators per bucket, zeroed
            Gsb = [cp.tile([P, HID], f32, tag=f"Gsb{b}", name=f"Gsb{b}")
                   for b in range(NBKT)]
            for b in range(NBKT):
                nc.gpsimd.memset(Gsb[b][:], 0.0)

            with tc.tile_pool(name="psR", bufs=1, space="PSUM") as psR, \
                 tc.tile_pool(name="xin", bufs=2) as xp, \
                 tc.tile_pool(name="act", bufs=3) as ap_, \
                 tc.tile_pool(name="sc", bufs=3) as sp:

                # PSUM ping-pong regions: [h1 (2x512) | a2 (4x256)] = 4 banks
                REG = [psR.tile([P, 8, 256], f32, tag=f"REG{x}",
                                name=f"REG{x}") for x in range(2)]

                # PE warm-up: ~4us of back-to-back matmuls flips the HAM
                # clock gate from 4/8 (1.2 GHz) to 8/8 (2.4 GHz); the loop's
                # filler matmuls then keep it warm. Targets the h1 area that
                # L1(0) overwrites, so no extra PSUM and no pool barrier.
                for _ in range(9):
                    nc.tensor.matmul(out=REG[0][:, 0:2, :],
                                     lhsT=wu_sb[:, 0, 0:P], rhs=wu_sb[:],
                                     start=True, stop=True,
                                     skip_group_check=True)

                xt_tiles = {}

                def ensure_xt(k):
                    """Emit the DMA for the batch containing superchunk k."""
                    nb = k // DMACH
                    if nb in xt_tiles or k >= nsup:
                        return
                    nsw = min(DMACH, nsup - nb * DMACH)
                    t = xp.tile([10, DMACH, SUP], f16, tag="xt")
                    nc.sync.dma_start(
                        out=t[:, :nsw, :],
                        in_=xtr_d[:, nb * DMACH * SUP:
                                  (nb * DMACH + nsw) * SUP])
                    xt_tiles[nb] = t
                    xt_tiles.pop(nb - 2, None)

                def emit_L1(k):
                    if k >= nsup:
                        return
                    ensure_xt(k)
                    r = k % 2
                    xt = xt_tiles[k // DMACH]
                    for m in range(2):
                        nc.tensor.matmul(
                            out=REG[r][:, 2 * m:2 * m + 2, :],
                            lhsT=W1f_t[:, m * P:(m + 1) * P],
                            rhs=xt[:, k % DMACH, :],
                            start=True, stop=True,
                            skip_group_check=True)

                def emit_DVE_scatter(ao, ksrc):
                    """Fold a2h(ksrc) (= ao[:, 4:8, :]) into Gsb[bucket]."""
                    b = kbkt[ksrc]
                    s1 = sp.tile([P, HID], f16, tag="s1")
                    nc.vector.tensor_tensor(out=s1[:], in0=ao[:, 4, :],
                                            in1=ao[:, 5, :],
                                            op=mybir.AluOpType.add)
                    s2 = sp.tile([P, HID], f16, tag="s2")
                    nc.vector.tensor_tensor(out=s2[:], in0=ao[:, 6, :],
                                            in1=ao[:, 7, :],
                                            op=mybir.AluOpType.add)
                    s3 = sp.tile([P, HID], f16, tag="s3")
                    nc.vector.tensor_tensor(out=s3[:], in0=s1[:], in1=s2[:],
                                            op=mybir.AluOpType.add)
                    s4 = sp.tile([P, HID], f32, tag="s4")
                    nc.vector.tensor_copy(out=s4[:], in_=s3[:])
                    nc.vector.tensor_tensor(out=Gsb[b][:], in0=Gsb[b][:],
                                            in1=s4[:],
                                            op=mybir.AluOpType.add)

                ndummy = int(os.environ.get("MAGNO_DUMMY", "2"))
                emit_L1(0)
                emit_L1(1)
                for k in range(nsup):
                    r = k % 2
                    # fused ACT(k): gelu over [h1(k) | a2(k-2)]
                    ao = ap_.tile([P, 8, 256], f16, tag="ao")
                    if k >= 2:
                        nc.scalar.activation(out=ao[:], in_=REG[r][:],
                                             func=GELU)
                    else:
                        nc.scalar.activation(out=ao[:, 0:4, :],
                                             in_=REG[r][:, 0:4, :], func=GELU)
                    # PE filler: dummy matmuls into the h1 area that L1
                    # overwrites right after; enough PE duty to keep the HAM
                    # clock gate at 8/8 (2.4 GHz).
                    for d in range(ndummy):
                        nc.tensor.matmul(out=REG[r][:, 0:2, :],
                                         lhsT=wu_sb[:, 0, 0:P],
                                         rhs=wu_sb[:],
                                         start=True, stop=True,
                                         skip_group_check=True)
                    # PE: L1(k+2) first, then L2(k) (keeps ACT chain fed)
                    emit_L1(k + 2)
                    # trigger the next input batch one iteration early so its
                    # ~2us DMA latency hides behind a full ACT period
                    ensure_xt(k + 3)
                    for j in range(NCH):
                        q = j // 2
                        o = (j % 2) * P
                        for m in range(2):
                            nc.tensor.matmul(
                                out=REG[r][:, 4 + j, :],
                                lhsT=ao[:, 2 * m + q, o:o + P],
                                rhs=W2v_t[:, m, :],
                                start=(m == 0), stop=(m == 1),
                                skip_group_check=True)
                    # DVE: scatter a2h(k-2)
                    if k >= 2:
                        emit_DVE_scatter(ao, k - 2)

                # tails: a2(nsup-2), a2(nsup-1) still in PSUM
                for ktail in (nsup - 2, nsup - 1):
                    r = ktail % 2
                    ao = ap_.tile([P, 8, 256], f16, tag="ao")
                    nc.scalar.activation(out=ao[:, 4:8, :],
                                         in_=REG[r][:, 4:8, :], func=GELU)
                    emit_DVE_scatter(ao, ktail)

            # ---- epilogue: O = (Gsb @ diag(rcnt))^T @ W3 (+ b3) ----
            with tc.tile_pool(name="ep", bufs=2) as ep, \
                 tc.tile_pool(name="psE", bufs=2, space="PSUM") as psE:
                if b3nz:
                    b3_ps = psE.tile([P, HID], f32, tag="b3bc")
                    nc.tensor.matmul(out=b3_ps[:], lhsT=ones1_t[:], rhs=b3_t[:],
                                     start=True, stop=True)
                    b3bc_t = ep.tile([P, HID], f32, tag="b3bc")
                    nc.vector.tensor_copy(out=b3bc_t[:], in_=b3_ps[:])
                for b in range(NBKT):
                    # gt[h, l] = Gsb[l, h] * rcnt[l] via matmul with diag
                    gsh_t = ep.tile([P, HID], f16, tag="gsh")
                    nc.vector.tensor_copy(out=gsh_t[:], in_=Gsb[b][:])
                    gth_t = ep.tile([P, 2, P], f16, tag="gth")
                    for q in range(2):
                        gt_ps = psE.tile([P, P], f32, tag="gt")
                        nc.tensor.matmul(out=gt_ps[:],
                                         lhsT=gsh_t[:, q * P:(q + 1) * P],
                                         rhs=rdiag_t[:, b, :],
                                         start=True, stop=True)
                        nc.vector.tensor_copy(out=gth_t[:, q, :], in_=gt_ps[:])
                    o_ps = psE.tile([P, HID], f32, tag="o")
                    nc.tensor.matmul(out=o_ps[:], lhsT=gth_t[:, 0, :],
                                     rhs=W3p_t[:, 0, :], start=True, stop=False)
                    nc.tensor.matmul(out=o_ps[:], lhsT=gth_t[:, 1, :],
                                     rhs=W3p_t[:, 1, :], start=False, stop=True)
                    o_t = ep.tile([P, HID], f32, tag="osb")
                    if b3nz:
                        nc.vector.tensor_tensor(out=o_t[:], in0=o_ps[:],
                                                in1=b3bc_t[:],
                                                op=mybir.AluOpType.add)
                    else:
                        nc.vector.tensor_copy(out=o_t[:], in_=o_ps[:])
                    nc.default_dma_engine.dma_start(
                        out=out_d[b * P:(b + 1) * P, :], in_=o_t[:])

    nc.finalize()
    return nc


def kernel(phys_feats, phys_pos, latent_pos, edge_src, edge_dst,
           W1, b1, W2, b2, W3, b3):
    global last_results
    phys_feats = np.asarray(phys_feats, dtype=np.float32)
    phys_pos = np.asarray(phys_pos, dtype=np.float32)
    latent_pos = np.asarray(latent_pos, dtype=np.float32)
    W1 = np.asarray(W1, dtype=np.float32)
    W2 = np.asarray(W2, dtype=np.float32)
    W3 = np.asarray(W3, dtype=np.float32)
    b1 = np.asarray(b1, dtype=np.float32)
    b2 = np.asarray(b2, dtype=np.float32)
    b3 = np.asarray(b3, dtype=np.float32)
    src_all = np.asarray(edge_src).reshape(-1).astype(np.int64)
    dst_all = np.asarray(edge_dst).reshape(-1).astype(np.int64)
    E = src_all.shape[0]
    assert not b1.any() and not b2.any(), "zero b1/b2 assumed (spec fill=zeros)"

    # ---- latent -> (core, bucket, slot) balanced assignment ----
    cnt_all = np.bincount(dst_all, minlength=N_LATENT)
    latorder = np.argsort(-cnt_all, kind="stable")  # descending count
    lat_core = np.zeros(N_LATENT, np.int64)
    lat_bkt = np.zeros(N_LATENT, np.int64)
    lat_slot = np.zeros(N_LATENT, np.int64)
    slice_max = np.zeros((NCORES, NBKT), np.int64)
    for s in range(NCORES * NBKT):
        b, rr = divmod(s, NCORES)
        c = rr if b % 2 == 0 else NCORES - 1 - rr
        lats = latorder[s * P:(s + 1) * P]
        lat_core[lats] = c
        lat_bkt[lats] = b
        lat_slot[lats] = np.arange(P)
        slice_max[c, b] = max(int(cnt_all[lats].max()), 1)

    # bucket b needs max over cores, rounded to whole superchunks
    supb = [int(-(-slice_max[:, b].max() // NCH)) for b in range(NBKT)]
    nsup = sum(supb)
    ne = nsup * SUP
    bktcols = [s * NCH for s in supb]
    bktcol0 = np.concatenate([[0], np.cumsum(bktcols)])  # column offset/bucket

    # ---- per-edge grid position ----
    order = np.argsort(dst_all, kind="stable")
    sdst = dst_all[order]
    ssrc = src_all[order]
    start = np.searchsorted(sdst, np.arange(N_LATENT))
    jrank = np.arange(E) - start[sdst]
    ecore = lat_core[sdst]
    epos = (bktcol0[lat_bkt[sdst]] + jrank) * P + lat_slot[sdst]

    X9 = np.concatenate(
        [phys_feats[ssrc], phys_pos[ssrc], latent_pos[sdst]],
        axis=1).astype(np.float32)  # [E, 9]

    W1p = np.concatenate([W1[0:3], W1[3:6] - W1[6:9], W1[6:9]], axis=0)
    W1f = np.zeros((10, HID), np.float32)
    W1f[:9] = W1p
    W1f_host = W1f.astype(np.float16)
    W2v = np.ascontiguousarray(
        W2.reshape(2, P, HID).transpose(1, 0, 2).reshape(P, 2 * HID)
    ).astype(np.float16)
    W3p = np.ascontiguousarray(
        W3.reshape(2, P, HID).transpose(1, 0, 2).reshape(P, 2 * HID)
    ).astype(np.float16)
    ones1 = np.ones((1, P), dtype=np.float32)
    b3nz = bool(b3.any())

    in_maps = []
    for c in range(NCORES):
        sel = ecore == c
        Xc = np.zeros((ne, 10), np.float32)
        Xc[epos[sel], :9] = X9[sel]
        xtr = np.ascontiguousarray(Xc.T).astype(np.float16)  # [10, ne]
        # rdiag[l, b, l'] = (l == l') / max(cnt, 1)
        rdiag = np.zeros((P, NBKT, P), np.float32)
        for b in range(NBKT):
            lats = np.where((lat_core == c) & (lat_bkt == b))[0]
            rc = np.zeros(P, np.float32)
            rc[lat_slot[lats]] = 1.0 / np.maximum(cnt_all[lats], 1)
            rdiag[np.arange(P), b, np.arange(P)] = rc
        in_maps.append(dict(
            xtr=xtr, W1f=W1f_host, W2v=W2v, W3p=W3p,
            rdiag=np.ascontiguousarray(
                rdiag.reshape(P, NBKT * P)).astype(np.float16),
            b3r=b3[None, :], ones1=ones1,
        ))

    nc = _build_program(supb, b3nz)
    trace = bool(int(os.environ.get("MAGNO_TRACE", "0")))
    ncores_run = int(os.environ.get("MAGNO_CORES", str(NCORES)))
    res = run_bass_kernel_spmd(nc, in_maps[:ncores_run],
                               core_ids=list(range(ncores_run)), trace=trace)
    last_results = res

    out = np.zeros((N_LATENT, HID), np.float32)
    for c in range(ncores_run):
        oc = np.asarray(res.results[c]["out"])  # [NBKT*P, HID]
        lats = np.where(lat_core == c)[0]
        out[lats] = oc[lat_bkt[lats] * P + lat_slot[lats]]
    return out
